# revision 1
# baseline (speedup 1.0000x reference)
"""LCSA (local convolutional sparse attention) Trainium2 Bass kernel.

Problem: B=2, S=2048, D=1024, H=8 heads, E=128 head width, KW=16 kernel width,
per-head dilations [1,1,2,2,4,4,8,8].

Sharding: pure data-parallel over (batch, sequence): core c handles batch c//4,
sequence chunk (c%4)*512..+512. Each core loads a 640-token haloed slice of x
(64-token halo each side, zero-padded at batch edges; padding reproduces the
reference's "invalid position -> bias" semantics exactly since k(0)=bk, v(0)=bv).

Device algorithm per core (all in fp32):
  - x arrives pre-transposed [D=1024, 640] (host does the transpose for free).
  - qT[h] = (Wq[h].T @ xT) [E,512] and kT[h] [E,640] via PE with W chunks
    stationary; v = xT.T @ Wv_allheads [640, H*E] with xT chunks stationary.
  - Per (query-tile i of 128, head h): logits = qT_tile.T @ kT_window -> [128,256]
    (full 256-key span; additive -30000 mask keeps only the 16 dilated window
    positions), softmax along free dim (DVE max / ACT exp+rowsum / DVE recip),
    score transposed via PE, attnT = v_span.T-chunks @ scoreT, out accumulated
    over heads: out[i] = sum_h attnT[h].T @ Wo[h] (Wo pre-scaled by E**-0.5).
"""

import numpy as np

B, S, D, H, E, KW = 2, 2048, 1024, 8, 128, 16
HALO = 64          # covers max offset d*(KW-1)//2 = 60 for d=8
CHUNK = 512        # query tokens per core
SPAN = CHUNK + 2 * HALO   # 640 = 5*128 kv tokens per core
NST = SPAN // 128  # 5 sequence tiles
NQT = CHUNK // 128 # 4 query tiles
NC_ = 8            # cores
DC = D // 128      # 8 contraction chunks
MASKVAL = -30000.0

_CACHE: dict = {}
_SMBUFS = 3
_SMVBUFS = 6
_PJBUFS = 2
_PSATBUFS = 1
_VFIRST = True
_ATBUFS = 3
_COPY_ENG = "vector"
_P1ENG = "scalar"


def _P1CP(nc, out, in_, bias):
    import concourse.mybir as _mb
    if _P1ENG == "scalar":
        nc.scalar.activation(out, in_, _mb.ActivationFunctionType.Identity,
                             bias=bias, scale=1.0)
    else:
        nc.vector.tensor_scalar_add(out, in_, bias)


def _CP(nc):
    return nc.scalar.copy if _COPY_ENG == "scalar" else nc.vector.tensor_copy


def _build_nc(reps=1, f32r=True):
    from contextlib import ExitStack

    import concourse.bacc as bacc
    import concourse.tile as tile
    from concourse import mybir
    from concourse.masks import make_identity

    F32 = mybir.dt.float32
    # float32r streams the moving matmul operand at 1 cycle/row (vs 4 for
    # plain fp32) once the free dim is >=256; numerically verified against
    # the fp32 reference below.
    FR = mybir.dt.float32r if f32r else F32
    AX = mybir.AxisListType.X
    AF = mybir.ActivationFunctionType

    nc = bacc.Bacc("TRN2", target_bir_lowering=False, debug=False, num_devices=1)

    xt_d = nc.dram_tensor("xt", [D, SPAN], FR, kind="ExternalInput").ap()
    wq_d = nc.dram_tensor("wq", [H, D, E], FR, kind="ExternalInput").ap()
    wk_d = nc.dram_tensor("wk", [H, D, E], FR, kind="ExternalInput").ap()
    wvr_d = nc.dram_tensor("wvr", [D, H * E], FR, kind="ExternalInput").ap()
    wos_d = nc.dram_tensor("wos", [H, E, D], FR, kind="ExternalInput").ap()
    mk_d = nc.dram_tensor("mk", [H, 128, 256], F32, kind="ExternalInput").ap()
    bqt_d = nc.dram_tensor("bqt", [E, H], F32, kind="ExternalInput").ap()
    bkt_d = nc.dram_tensor("bkt", [E, H], F32, kind="ExternalInput").ap()
    bvr_d = nc.dram_tensor("bvr", [1, H * E], FR, kind="ExternalInput").ap()
    bor_d = nc.dram_tensor("bor", [1, D], FR, kind="ExternalInput").ap()
    ones_d = nc.dram_tensor("onesr", [1, 128], FR, kind="ExternalInput").ap()
    out_d = nc.dram_tensor("out", [CHUNK, D], F32, kind="ExternalOutput").ap()

    with tile.TileContext(nc) as tc, ExitStack() as ctx:
        const_p = ctx.enter_context(tc.tile_pool(name="const", bufs=1))
        big_p = ctx.enter_context(tc.tile_pool(name="big", bufs=1))
        wqk_p = ctx.enter_context(tc.tile_pool(name="wqk", bufs=2))
        sm_p = ctx.enter_context(tc.tile_pool(name="sm", bufs=_SMBUFS))
        smv_p = ctx.enter_context(tc.tile_pool(name="smv", bufs=_SMVBUFS))
        at_p = ctx.enter_context(tc.tile_pool(name="atsb", bufs=_ATBUFS))
        ob_p = ctx.enter_context(tc.tile_pool(name="ob", bufs=2))
        ps_pj = ctx.enter_context(tc.tile_pool(name="ps_pj", bufs=_PJBUFS, space="PSUM"))
        ps_lg = ctx.enter_context(tc.tile_pool(name="ps_lg", bufs=2, space="PSUM"))
        ps_st = ctx.enter_context(tc.tile_pool(name="ps_st", bufs=1, space="PSUM"))
        ps_at = ctx.enter_context(tc.tile_pool(name="ps_at", bufs=_PSATBUFS, space="PSUM"))
        ps_ou = ctx.enter_context(tc.tile_pool(name="ps_ou", bufs=2, space="PSUM"))

        # constants
        ident = const_p.tile([128, 128], F32)
        make_identity(nc, ident)
        ones = const_p.tile([1, 128], FR)
        nc.sync.dma_start(ones, ones_d)

        for _rep in range(reps):
            # resident loads
            xt_sb = big_p.tile([128, DC, SPAN], FR, tag="xt")
            nc.sync.dma_start(xt_sb, xt_d.rearrange("(c p) s -> p c s", p=128))
            bqt_sb = big_p.tile([128, H], F32, tag="bqt")
            nc.sync.dma_start(bqt_sb, bqt_d)
            bkt_sb = big_p.tile([128, H], F32, tag="bkt")
            nc.sync.dma_start(bkt_sb, bkt_d)
            bvr_sb = big_p.tile([1, H * E], FR, tag="bvr")
            nc.sync.dma_start(bvr_sb, bvr_d)
            bor_sb = big_p.tile([1, D], FR, tag="bor")
            nc.sync.dma_start(bor_sb, bor_d)
            mk_sb = big_p.tile([128, H, 256], F32, tag="mk")
            nc.sync.dma_start(mk_sb, mk_d.rearrange("h p t -> p h t"))
            wvr_sb = big_p.tile([128, DC, H * E], FR, tag="wvr")
            nc.sync.dma_start(wvr_sb, wvr_d.rearrange("(c p) n -> p c n", p=128))
            wos_sb = big_p.tile([128, H, D], FR, tag="wos")
            nc.sync.dma_start(wos_sb, wos_d.rearrange("h e d -> e h d"))

            # persistent projection outputs
            qT_sb = big_p.tile([128, H, CHUNK], FR, tag="qT")   # [e, h, s]
            kT_sb = big_p.tile([128, H, SPAN], FR, tag="kT")    # [e, h, s]
            v_sb = big_p.tile([128, NST, H * E], FR, tag="v")   # [s, tile, h*E+e]

            _emit_body(nc, tc, mybir, F32, FR, AX, AF,
                       wq_d, wk_d, out_d, wqk_p, sm_p, smv_p, at_p, ob_p,
                       ps_pj, ps_lg, ps_st, ps_at, ps_ou,
                       ident, ones, xt_sb, bqt_sb, bkt_sb, bvr_sb, bor_sb,
                       mk_sb, wvr_sb, wos_sb, qT_sb, kT_sb, v_sb)

    nc.compile()
    return nc


def _emit_v(nc, ps_pj, xt_sb, wvr_sb, bvr_sb, ones, v_sb, F32):
    # v projection, all heads at once (xT chunks stationary)
    for j in range(NST):
        for half in range(2):
            vp = ps_pj.tile([128, 512], F32, tag="pj")
            nsl = slice(512 * half, 512 * (half + 1))
            for c in range(DC):
                nc.tensor.matmul(vp, xt_sb[:, c, 128 * j:128 * (j + 1)],
                                 wvr_sb[:, c, nsl], start=(c == 0), stop=False)
            nc.tensor.matmul(vp, ones, bvr_sb[:, nsl], start=False, stop=True)
            if _P1ENG == "scalar":
                nc.scalar.copy(v_sb[:, j, nsl], vp)
            else:
                nc.vector.tensor_copy(v_sb[:, j, nsl], vp)


def _emit_body(nc, tc, mybir, F32, FR, AX, AF,
               wq_d, wk_d, out_d, wqk_p, sm_p, smv_p, at_p, ob_p,
               ps_pj, ps_lg, ps_st, ps_at, ps_ou,
               ident, ones, xt_sb, bqt_sb, bkt_sb, bvr_sb, bor_sb,
               mk_sb, wvr_sb, wos_sb, qT_sb, kT_sb, v_sb):
        if _VFIRST:
            _emit_v(nc, ps_pj, xt_sb, wvr_sb, bvr_sb, ones, v_sb, F32)
        # ---- phase 1a: q/k projections per head (W chunks stationary) ----
        for h in range(H):
            wq_sb = wqk_p.tile([128, DC, E], FR, tag="wq")
            nc.sync.dma_start(wq_sb, wq_d[h].rearrange("(c p) e -> p c e", p=128))
            wk_sb = wqk_p.tile([128, DC, E], FR, tag="wk")
            nc.sync.dma_start(wk_sb, wk_d[h].rearrange("(c p) e -> p c e", p=128))

            qp = ps_pj.tile([128, 512], F32, tag="pj")
            for c in range(DC):
                nc.tensor.matmul(qp, wq_sb[:, c, :], xt_sb[:, c, HALO:HALO + CHUNK],
                                 start=(c == 0), stop=(c == DC - 1))
            _P1CP(nc, qT_sb[:, h, :], qp, bqt_sb[:, h:h + 1])

            for half in range(2):
                kp = ps_pj.tile([128, 512], F32, tag="pj")
                sl = slice(320 * half, 320 * (half + 1))
                for c in range(DC):
                    nc.tensor.matmul(kp[:, 0:320], wk_sb[:, c, :], xt_sb[:, c, sl],
                                     start=(c == 0), stop=(c == DC - 1))
                _P1CP(nc, kT_sb[:, h, sl], kp[:, 0:320], bkt_sb[:, h:h + 1])

        if not _VFIRST:
            _emit_v(nc, ps_pj, xt_sb, wvr_sb, bvr_sb, ones, v_sb, F32)

        # ---- phase 2: attention + output projection per query tile ----
        for i in range(NQT):
            ou0 = ps_ou.tile([128, 512], F32, tag="ou")
            ou1 = ps_ou.tile([128, 512], F32, tag="ou")
            for h in range(H):
                lg = ps_lg.tile([128, 256], F32, tag="lg")
                nc.tensor.matmul(lg, qT_sb[:, h, 128 * i:128 * (i + 1)],
                                 kT_sb[:, h, 128 * i:128 * i + 256],
                                 start=True, stop=True)
                lm = sm_p.tile([128, 256], F32, tag="lm")
                nc.vector.tensor_add(lm, lg, mk_sb[:, h, :])
                nm = smv_p.tile([128, 1], F32, tag="nm")
                nc.vector.reduce_max(nm, lm, axis=AX, negate=True)
                ex = sm_p.tile([128, 256], F32, tag="ex")
                se = smv_p.tile([128, 1], F32, tag="se")
                nc.scalar.activation(ex, lm, AF.Exp, bias=nm, scale=1.0, accum_out=se)
                rc = smv_p.tile([128, 1], F32, tag="rc")
                nc.vector.reciprocal(rc, se)
                sc = sm_p.tile([128, 256], F32, tag="sc")
                nc.vector.tensor_scalar_mul(sc, ex, rc)

                st = ps_st.tile([128, 256], F32, tag="st")
                nc.tensor.transpose(st[:, 0:128], sc[:, 0:128], ident)
                nc.tensor.transpose(st[:, 128:256], sc[:, 128:256], ident)
                sct = sm_p.tile([128, 256], FR, tag="sct")
                _CP(nc)(sct, st)

                at = ps_at.tile([128, 128], F32, tag="at")
                nc.tensor.matmul(at, v_sb[:, i, E * h:E * (h + 1)], sct[:, 0:128],
                                 start=True, stop=False)
                nc.tensor.matmul(at, v_sb[:, i + 1, E * h:E * (h + 1)], sct[:, 128:256],
                                 start=False, stop=True)
                ats = at_p.tile([128, 128], FR, tag="ats")
                _CP(nc)(ats, at)

                nc.tensor.matmul(ou0, ats, wos_sb[:, h, 0:512],
                                 start=(h == 0), stop=False)
                nc.tensor.matmul(ou1, ats, wos_sb[:, h, 512:1024],
                                 start=(h == 0), stop=False)
            nc.tensor.matmul(ou0, ones, bor_sb[:, 0:512], start=False, stop=True)
            nc.tensor.matmul(ou1, ones, bor_sb[:, 512:1024], start=False, stop=True)

            ob = ob_p.tile([128, D], F32, tag="ob")
            nc.scalar.copy(ob[:, 0:512], ou0)
            nc.scalar.copy(ob[:, 512:1024], ou1)
            nc.sync.dma_start(out_d[128 * i:128 * (i + 1), :], ob)


def _host_prep(x, Wq, bq, Wk, bk, Wv, bv, Wo, bo, dilations):
    f = np.float32
    x = np.asarray(x, f)
    x_pad = np.zeros((B, S + 2 * HALO, D), f)
    x_pad[:, HALO:HALO + S] = x

    wvr = np.ascontiguousarray(
        np.asarray(Wv, f).transpose(1, 0, 2).reshape(D, H * E))
    wos = np.ascontiguousarray(np.asarray(Wo, f) * np.float32(E) ** f(-0.5))
    bqt = np.ascontiguousarray(np.asarray(bq, f).T)      # [E, H]
    bkt = np.ascontiguousarray(np.asarray(bk, f).T)
    bvr = np.ascontiguousarray(np.asarray(bv, f).reshape(1, H * E))
    bor = np.ascontiguousarray(np.asarray(bo, f).reshape(1, D))

    dil = np.asarray(dilations).astype(np.int64)
    masks = np.full((H, 128, 256), MASKVAL, f)
    s_i = np.arange(128)[:, None]
    t_i = np.arange(256)[None, :]
    for h in range(H):
        d = int(dil[h])
        off = (d * (KW - 1)) // 2
        delta = t_i - s_i - HALO + off
        win = (delta >= 0) & (delta <= (KW - 1) * d) & (delta % d == 0)
        masks[h][win] = 0.0

    shared = {
        "wq": np.ascontiguousarray(np.asarray(Wq, f)),
        "wk": np.ascontiguousarray(np.asarray(Wk, f)),
        "wvr": wvr, "wos": wos, "mk": masks,
        "bqt": bqt, "bkt": bkt, "bvr": bvr, "bor": bor,
        "onesr": np.ones((1, 128), f),
    }
    in_maps = []
    for c in range(NC_):
        b, idx = divmod(c, 4)
        xt = np.ascontiguousarray(x_pad[b, idx * CHUNK: idx * CHUNK + SPAN].T)
        in_maps.append({"xt": xt, **shared})
    return in_maps


def kernel(x, Wq, bq, Wk, bk, Wv, bv, Wo, bo, dilations):
    from concourse.bass_utils import run_bass_kernel_spmd

    if "nc" not in _CACHE:
        _CACHE["nc"] = _build_nc()
    nc = _CACHE["nc"]

    in_maps = _host_prep(x, Wq, bq, Wk, bk, Wv, bv, Wo, bo, dilations)
    res = run_bass_kernel_spmd(nc, in_maps, core_ids=list(range(NC_)))

    out = np.empty((B, S, D), np.float32)
    for c in range(NC_):
        b, idx = divmod(c, 4)
        out[b, idx * CHUNK:(idx + 1) * CHUNK] = res.results[c]["out"]
    return out



# revision 6
# speedup vs baseline: 1.4782x; 1.4782x over previous
"""LCSA (local convolutional sparse attention) Trainium2 Bass kernel.

Problem: B=2, S=2048, D=1024, H=8 heads, E=128 head width, KW=16 kernel width,
per-head dilations [1,1,2,2,4,4,8,8].

Sharding: data-parallel over (batch, sequence): core c handles batch c//4,
sequence chunk (c%4)*512..+512, with a 64-token zero-padded halo per side.

Device algorithm per core (fp32 q/k path, bf16 value path):
  - qT[h] = Wq[h].T @ xT [E,512]; kT[h] [E,640] (fp32r, PE).  k-bias dropped
    (uniform per-query logit shift -> softmax invariant); q-bias via ACT copy.
  - v = xTb.T @ Wv_allheads [640, H*E] in bf16 (xt cast to bf16 on Pool).
    v-bias and out-bias folded into a host-side constant (sum of scores = 1).
  - Per (query tile i, head h): PSUM logits = mask (PE identity-matmul
    preload, in-window value -40 to bound exp) + qT_i.T @ kT window [128,256];
    exp+rowsum on ACT (no max subtraction; |logit|<=81 so exp(l-40) is safe);
    reciprocal on DVE; normalize on Pool (bf16); transpose via PE (bf16);
    attnT = v.T @ scoreT (bf16); out_i = sum_h attnT.T @ Wo[h] (bf16 moving).
  - Software-pipelined emission keeps all five engines concurrently busy;
    warm-up matmuls ramp the PE p-state before real data lands.
"""

import numpy as np

B, S, D, H, E, KW = 2, 2048, 1024, 8, 128, 16
HALO = 64          # covers max offset d*(KW-1)//2 = 60 for d=8
CHUNK = 512        # query tokens per core
SPAN = CHUNK + 2 * HALO   # 640 kv tokens per core
NST = SPAN // 128  # 5 sequence tiles
NQT = CHUNK // 128 # 4 query tiles
NC_ = 8            # cores
DC = D // 128      # 8 contraction chunks
NT = NQT * H       # 32 attention tiles per core
MASKVAL = -30000.0
SHIFT = -40.0      # in-window logit shift; bounds exp while leaving softmax exact

_CACHE: dict = {}
N_WARM = 14        # PE warm-up matmuls (p-state ramp + DMA-latency cover)


def _build_nc(reps=1, f32r=True):
    from contextlib import ExitStack

    import concourse.bacc as bacc
    import concourse.tile as tile
    from concourse import mybir
    from concourse.masks import make_identity

    F32 = mybir.dt.float32
    BF16 = mybir.dt.bfloat16
    FR = mybir.dt.float32r if f32r else F32
    AF = mybir.ActivationFunctionType

    nc = bacc.Bacc("TRN2", target_bir_lowering=False, debug=False, num_devices=1)

    xt_d = nc.dram_tensor("xt", [D, SPAN], FR, kind="ExternalInput").ap()
    wq_d = nc.dram_tensor("wq", [H, D, E], FR, kind="ExternalInput").ap()
    wk_d = nc.dram_tensor("wk", [H, D, E], FR, kind="ExternalInput").ap()
    wvrb_d = nc.dram_tensor("wvrb", [D, H * E], BF16, kind="ExternalInput").ap()
    wosb_d = nc.dram_tensor("wosb", [H, E, D], BF16, kind="ExternalInput").ap()
    mkb_d = nc.dram_tensor("mkb", [H, 128, 256], BF16, kind="ExternalInput").ap()
    bqt_d = nc.dram_tensor("bqt", [E, H], F32, kind="ExternalInput").ap()
    out_d = nc.dram_tensor("out", [CHUNK, D], F32, kind="ExternalOutput").ap()

    with tile.TileContext(nc) as tc, ExitStack() as ctx:
        const_p = ctx.enter_context(tc.tile_pool(name="const", bufs=1))
        big_s = ctx.enter_context(tc.tile_pool(name="bigs", bufs=1))
        wring = ctx.enter_context(tc.tile_pool(name="wring", bufs=4))
        sm_p = ctx.enter_context(tc.tile_pool(name="sm", bufs=3))
        smv_p = ctx.enter_context(tc.tile_pool(name="smv", bufs=4))
        ob_p = ctx.enter_context(tc.tile_pool(name="ob", bufs=2))
        # PSUM: 8 banks exactly: big(3) + lg(3) + st(1) + at(1)
        ps_big = ctx.enter_context(tc.tile_pool(name="ps_big", bufs=3, space="PSUM"))
        ps_lg = ctx.enter_context(tc.tile_pool(name="ps_lg", bufs=3, space="PSUM"))
        ps_st = ctx.enter_context(tc.tile_pool(name="ps_st", bufs=1, space="PSUM"))
        ps_at = ctx.enter_context(tc.tile_pool(name="ps_at", bufs=1, space="PSUM"))

        # ---- constants (Pool-generated; no DMA dependency) ----
        identb = const_p.tile([128, 128], BF16)
        make_identity(nc, identb)
        warmb = const_p.tile([128, 512], BF16)
        nc.gpsimd.memset(warmb, 0.0)

        for _rep in range(reps):
            _emit(nc, tc, mybir, F32, BF16, FR, AF,
                  xt_d, wq_d, wk_d, wvrb_d, wosb_d, mkb_d, bqt_d, out_d,
                  const_p, big_s, wring, sm_p, smv_p, ob_p,
                  ps_big, ps_lg, ps_st, ps_at, identb, warmb)

    nc.compile()
    return nc


def _emit(nc, tc, mybir, F32, BF16, FR, AF,
          xt_d, wq_d, wk_d, wvrb_d, wosb_d, mkb_d, bqt_d, out_d,
          const_p, big_s, wring, sm_p, smv_p, ob_p,
          ps_big, ps_lg, ps_st, ps_at, identb, warmb):
    # ---- resident loads, ordered by first PE use ----
    wq0 = wring.tile([128, DC, E], FR, tag="wq", name="wq0")
    nc.sync.dma_start(wq0, wq_d[0].rearrange("(c p) e -> p c e", p=128))
    wk0 = wring.tile([128, DC, E], FR, tag="wk", name="wk0")
    nc.sync.dma_start(wk0, wk_d[0].rearrange("(c p) e -> p c e", p=128))

    xt_sb = big_s.tile([128, DC, SPAN], FR, tag="xt")
    for c in range(DC):
        nc.sync.dma_start(xt_sb[:, c, :], xt_d[128 * c:128 * (c + 1), :])
    bqt_sb = big_s.tile([128, H], F32, tag="bqt")
    nc.sync.dma_start(bqt_sb, bqt_d)

    wvrb_sb = big_s.tile([128, DC, H * E], BF16, tag="wvrb")
    nc.sync.dma_start(wvrb_sb[:, :, 0:512],
                      wvrb_d[:, 0:512].rearrange("(c p) n -> p c n", p=128))

    w_ring = {0: (wq0, wk0)}
    def _load_head(h):
        wqh = wring.tile([128, DC, E], FR, tag="wq", name=f"wq{h}")
        nc.sync.dma_start(wqh, wq_d[h].rearrange("(c p) e -> p c e", p=128))
        wkh = wring.tile([128, DC, E], FR, tag="wk", name=f"wk{h}")
        nc.sync.dma_start(wkh, wk_d[h].rearrange("(c p) e -> p c e", p=128))
        w_ring[h] = (wqh, wkh)

    _load_head(1)
    nc.sync.dma_start(wvrb_sb[:, :, 512:1024],
                      wvrb_d[:, 512:1024].rearrange("(c p) n -> p c n", p=128))
    _load_head(2)
    _load_head(3)
    mkb_sb = big_s.tile([128, H, 256], BF16, tag="mkb")
    nc.sync.dma_start(mkb_sb, mkb_d.rearrange("h p t -> p h t"))
    wosb_sb = big_s.tile([128, H, D], BF16, tag="wosb")
    nc.sync.dma_start(wosb_sb, wosb_d.rearrange("h e d -> e h d"))
    for h in range(4, H):
        _load_head(h)

    # ---- persistent projection outputs ----
    qT_sb = big_s.tile([128, H, CHUNK], FR, tag="qT")    # [e, h, s]
    kT_sb = big_s.tile([128, H, SPAN], FR, tag="kT")     # [e, h, s]
    xtb_sb = big_s.tile([128, DC, SPAN], BF16, tag="xtb")
    vb_sb = big_s.tile([128, NST, H * E], BF16, tag="vb")  # [s, tile, h*E+e]

    # ---- PE warm-up: ramp p-state while DMAs stream ----
    for w in range(N_WARM):
        wp = ps_big.tile([128, 512], F32, tag="big", name=f"warm{w}")
        nc.tensor.matmul(wp, identb, warmb, start=True, stop=True)

    # ---- xt -> bf16 cast on Pool (for the v projection) ----
    for c in range(DC):
        nc.gpsimd.tensor_copy(xtb_sb[:, c, :], xt_sb[:, c, :])

    # ---- phase 1: projections ----
    def _qk(h):
        wqh, wkh = w_ring[h]
        qp = ps_big.tile([128, 512], F32, tag="big", name=f"qp{h}")
        for c in range(DC):
            nc.tensor.matmul(qp, wqh[:, c, :], xt_sb[:, c, HALO:HALO + CHUNK],
                             start=(c == 0), stop=(c == DC - 1))
        nc.scalar.activation(qT_sb[:, h, :], qp, mybir.ActivationFunctionType.Identity,
                             bias=bqt_sb[:, h:h + 1], scale=1.0)
        for half in range(2):
            kp = ps_big.tile([128, 512], F32, tag="big", name=f"kp{h}_{half}")
            sl = slice(320 * half, 320 * (half + 1))
            for c in range(DC):
                nc.tensor.matmul(kp[:, 0:320], wkh[:, c, :], xt_sb[:, c, sl],
                                 start=(c == 0), stop=(c == DC - 1))
            nc.scalar.copy(kT_sb[:, h, sl], kp[:, 0:320])

    def _vhalf(half):
        nsl = slice(512 * half, 512 * (half + 1))
        for j in range(NST):
            vp = ps_big.tile([128, 512], F32, tag="big", name=f"vp{half}_{j}")
            for c in range(DC):
                nc.tensor.matmul(vp, xtb_sb[:, c, 128 * j:128 * (j + 1)],
                                 wvrb_sb[:, c, nsl], start=(c == 0), stop=(c == DC - 1))
            nc.vector.tensor_copy(vb_sb[:, j, nsl], vp)

    _qk(0)
    _vhalf(0)
    _qk(1)
    _vhalf(1)
    for h in range(2, H):
        _qk(h)

    # ---- phase 2: attention, software pipelined ----
    # slot u emits PE: tr(u-1), at(u-1), op(u-2), m(u+2)+l(u+2)
    #            DVE: recip(u), sct(u-2), ats(u-2)
    #            Pool: mul(u)
    #            ACT: exp(u) (own pace), ob(i) after op(i*8+7)
    lg_t, ex_t, se_t, rc_t, sc_t, st_t, sct_t, at_t, ats_t = ({} for _ in range(9))
    ou_t, ob_t = {}, {}

    def e_lg(t):
        i, h = divmod(t, 8)
        lg = ps_lg.tile([128, 256], F32, tag="lg", name=f"lg{t}")
        lg_t[t] = lg
        nc.tensor.matmul(lg, identb, mkb_sb[:, h, :], start=True, stop=False)
        nc.tensor.matmul(lg, qT_sb[:, h, 128 * i:128 * (i + 1)],
                         kT_sb[:, h, 128 * i:128 * i + 256],
                         start=False, stop=True)

    def e_exp(t):
        ex = sm_p.tile([128, 256], BF16, tag="ex", name=f"ex{t}")
        se = smv_p.tile([128, 1], F32, tag="se", name=f"se{t}")
        nc.scalar.activation(ex, lg_t.pop(t), AF.Exp, bias=0.0, scale=1.0,
                             accum_out=se)
        ex_t[t], se_t[t] = ex, se

    def e_recip(t):
        rc = smv_p.tile([128, 1], F32, tag="rc", name=f"rc{t}")
        nc.vector.reciprocal(rc, se_t.pop(t))
        rc_t[t] = rc

    def e_mul(t):
        sc = sm_p.tile([128, 256], BF16, tag="sc", name=f"sc{t}")
        nc.gpsimd.tensor_scalar_mul(sc, ex_t.pop(t), rc_t.pop(t))
        sc_t[t] = sc

    def e_tr(t):
        st = ps_st.tile([128, 256], BF16, tag="st", name=f"st{t}")
        sc = sc_t.pop(t)
        nc.tensor.transpose(st[:, 0:128], sc[:, 0:128], identb)
        nc.tensor.transpose(st[:, 128:256], sc[:, 128:256], identb)
        st_t[t] = st

    def e_sct(t):
        sct = sm_p.tile([128, 256], BF16, tag="sct", name=f"sct{t}")
        nc.vector.tensor_copy(sct, st_t.pop(t))
        sct_t[t] = sct

    def e_at(t):
        i = t // 8
        h = t % 8
        at = ps_at.tile([128, 128], F32, tag="at", name=f"at{t}")
        sct = sct_t.pop(t)
        nc.tensor.matmul(at, vb_sb[:, i, E * h:E * (h + 1)], sct[:, 0:128],
                         start=True, stop=False)
        nc.tensor.matmul(at, vb_sb[:, i + 1, E * h:E * (h + 1)], sct[:, 128:256],
                         start=False, stop=True)
        at_t[t] = at

    def e_ats(t):
        ats = sm_p.tile([128, 128], BF16, tag="ats", name=f"ats{t}")
        nc.vector.tensor_copy(ats, at_t.pop(t))
        ats_t[t] = ats

    def e_op(t):
        i, h = divmod(t, 8)
        if h == 0:
            ou0 = ps_big.tile([128, 512], F32, tag="big", name=f"ou0_{i}")
            ou1 = ps_big.tile([128, 512], F32, tag="big", name=f"ou1_{i}")
            ou_t[i] = (ou0, ou1)
        ou0, ou1 = ou_t[i]
        ats = ats_t.pop(t)
        nc.tensor.matmul(ou0, ats, wosb_sb[:, h, 0:512],
                         start=(h == 0), stop=(h == 7))
        nc.tensor.matmul(ou1, ats, wosb_sb[:, h, 512:1024],
                         start=(h == 0), stop=(h == 7))

    def e_ob(i):
        ou0, ou1 = ou_t.pop(i)
        ob = ob_p.tile([128, D], F32, tag="ob", name=f"ob{i}")
        nc.scalar.copy(ob[:, 0:512], ou0)
        nc.scalar.copy(ob[:, 512:1024], ou1)
        nc.sync.dma_start(out_d[128 * i:128 * (i + 1), :], ob)

    # pipeline emission, slot u: PE [tr(u-2), at(u-3), op(u-4), lg(u+2)],
    # ACT [exp(u), ob], DVE [sct(u-2), ats(u-3), recip(u)], Pool [mul(u)].
    # Chain lg(t)->exp(t)->recip(t)->mul(t) finishes mid slot t+1; tr(t) runs
    # slot t+2, so PE never waits on the softmax chain in steady state.
    e_lg(0)
    e_lg(1)
    for u in range(NT + 6):
        if 0 <= u - 2 < NT:
            e_tr(u - 2)
        if 0 <= u - 3 < NT:
            e_at(u - 3)
        if 0 <= u - 4 < NT:
            e_op(u - 4)
        if u + 2 < NT:
            e_lg(u + 2)
        if u < NT:
            e_exp(u)
        if u >= 11 and (u - 11) % 8 == 0 and (u - 11) // 8 < NQT:
            e_ob((u - 11) // 8)
        if 0 <= u - 2 < NT:
            e_sct(u - 2)
        if 0 <= u - 3 < NT:
            e_ats(u - 3)
        if u < NT:
            e_recip(u)
            e_mul(u)


def _host_prep(x, Wq, bq, Wk, bk, Wv, bv, Wo, bo, dilations):
    import ml_dtypes
    f = np.float32
    bf = ml_dtypes.bfloat16
    x = np.asarray(x, f)
    x_pad = np.zeros((B, S + 2 * HALO, D), f)
    x_pad[:, HALO:HALO + S] = x

    Wo_s = np.asarray(Wo, f) * np.float32(E) ** f(-0.5)
    wvrb = np.ascontiguousarray(
        np.asarray(Wv, f).transpose(1, 0, 2).reshape(D, H * E)).astype(bf)
    wosb = np.ascontiguousarray(Wo_s).astype(bf)
    bqt = np.ascontiguousarray(np.asarray(bq, f).T)      # [E, H]

    # host-folded constant: sum_h (bv_h/sqrt(E)) @ Wo_h + bo  (sum of scores = 1)
    hostc = np.einsum('he,hed->d', np.asarray(bv, f) * np.float32(E) ** f(-0.5),
                      np.asarray(Wo, f)) + np.asarray(bo, f)

    dil = np.asarray(dilations).astype(np.int64)
    masks = np.full((H, 128, 256), MASKVAL, f)
    s_i = np.arange(128)[:, None]
    t_i = np.arange(256)[None, :]
    for h in range(H):
        d = int(dil[h])
        off = (d * (KW - 1)) // 2
        delta = t_i - s_i - HALO + off
        win = (delta >= 0) & (delta <= (KW - 1) * d) & (delta % d == 0)
        masks[h][win] = SHIFT
    mkb = masks.astype(bf)

    shared = {
        "wq": np.ascontiguousarray(np.asarray(Wq, f)),
        "wk": np.ascontiguousarray(np.asarray(Wk, f)),
        "wvrb": wvrb, "wosb": wosb, "mkb": mkb, "bqt": bqt,
    }
    in_maps = []
    for c in range(NC_):
        b, idx = divmod(c, 4)
        xt = np.ascontiguousarray(x_pad[b, idx * CHUNK: idx * CHUNK + SPAN].T)
        in_maps.append({"xt": xt, **shared})
    return in_maps, hostc


def kernel(x, Wq, bq, Wk, bk, Wv, bv, Wo, bo, dilations):
    from concourse.bass_utils import run_bass_kernel_spmd

    if "nc" not in _CACHE:
        _CACHE["nc"] = _build_nc()
    nc = _CACHE["nc"]

    in_maps, hostc = _host_prep(x, Wq, bq, Wk, bk, Wv, bv, Wo, bo, dilations)
    res = run_bass_kernel_spmd(nc, in_maps, core_ids=list(range(NC_)))

    out = np.empty((B, S, D), np.float32)
    for c in range(NC_):
        b, idx = divmod(c, 4)
        out[b, idx * CHUNK:(idx + 1) * CHUNK] = res.results[c]["out"]
    out += hostc[None, None, :]
    return out


# revision 37
# speedup vs baseline: 1.5748x; 1.0653x over previous
"""LCSA (local convolutional sparse attention) Trainium2 Bass kernel.

Problem: B=2, S=2048, D=1024, H=8 heads, E=128 head width, KW=16 kernel width,
per-head dilations [1,1,2,2,4,4,8,8].

Sharding: data-parallel over (batch, sequence): core c handles batch c//4,
sequence chunk (c%4)*512..+512, with a 64-token zero-padded halo per side.

Device algorithm per core (fp32 q/k path, bf16 value path):
  - qT[h] = Wq[h].T @ xT [E,512]; kT[h] [E,640] (fp32r, PE).  k-bias dropped
    (uniform per-query logit shift -> softmax invariant); q-bias via ACT copy.
  - v = xTb.T @ Wv_allheads [640, H*E] in bf16 (xt cast to bf16 on Pool).
    v-bias and out-bias folded into a host-side constant (sum of scores = 1).
  - Per (query tile i, head h): PSUM logits = mask (PE identity-matmul
    preload, in-window value -40 to bound exp) + qT_i.T @ kT window [128,256];
    exp+rowsum on ACT (no max subtraction; |logit|<=81 so exp(l-40) is safe);
    reciprocal on DVE; normalize on Pool (bf16); transpose via PE (bf16);
    attnT = v.T @ scoreT (bf16); out_i = sum_h attnT.T @ Wo[h] (bf16 moving).
  - Software-pipelined emission keeps all five engines concurrently busy;
    warm-up matmuls ramp the PE p-state before real data lands.
"""

import numpy as np

B, S, D, H, E, KW = 2, 2048, 1024, 8, 128, 16
HALO = 64          # covers max offset d*(KW-1)//2 = 60 for d=8
CHUNK = 512        # query tokens per core
SPAN = CHUNK + 2 * HALO   # 640 kv tokens per core
NST = SPAN // 128  # 5 sequence tiles
NQT = CHUNK // 128 # 4 query tiles
NC_ = 8            # cores
DC = D // 128      # 8 contraction chunks
NT = NQT * H       # 32 attention tiles per core
MASKVAL = -30000.0
SHIFT = -40.0      # in-window logit shift; bounds exp while leaving softmax exact

_CACHE: dict = {}
N_WARM = 8         # PE warm-up matmuls (p-state ramp + DMA-latency cover)
QK0_FILL = 7       # filler matmuls between head-0 q chunks (xt DMA pacing)
DILATIONS = (1, 1, 2, 2, 4, 4, 8, 8)
# per-head kv span (in 640-wide span coords) actually reachable by the windows
K_SPANS = tuple((HALO - (15 * d) // 2, HALO + CHUNK + 15 * d - (15 * d) // 2)
                for d in DILATIONS)
# per-head logits window width from 128*i (span coords), multiple of 8, <=256
W_H = tuple(min(256, (HALO + 128 + 15 * d - (15 * d) // 2 + 7) // 8 * 8)
            for d in DILATIONS)


def _build_nc(reps=1, f32r=True):
    from contextlib import ExitStack

    import concourse.bacc as bacc
    import concourse.tile as tile
    from concourse import mybir
    from concourse.masks import make_identity

    F32 = mybir.dt.float32
    BF16 = mybir.dt.bfloat16
    FP16 = mybir.dt.float16
    FR = mybir.dt.float32r if f32r else F32
    AF = mybir.ActivationFunctionType

    nc = bacc.Bacc("TRN2", target_bir_lowering=False, debug=False, num_devices=1)

    # q/k path in fp16 (4x finer mantissa than bf16; halves the weight DMA).
    # wq/wk pre-rearranged on host to [H, 128, DC*E] so DMA rows stay >=512B.
    xt_d = nc.dram_tensor("xt", [D, SPAN], FP16, kind="ExternalInput").ap()
    wq_d = nc.dram_tensor("wq", [H, 128, DC * E], FP16, kind="ExternalInput").ap()
    wk_d = nc.dram_tensor("wk", [H, 128, DC * E], FP16, kind="ExternalInput").ap()
    wvrb_d = nc.dram_tensor("wvrb", [D, H * E], BF16, kind="ExternalInput").ap()
    wosb_d = nc.dram_tensor("wosb", [H, E, D], BF16, kind="ExternalInput").ap()
    mkb_d = nc.dram_tensor("mkb", [H, 128, 256], BF16, kind="ExternalInput").ap()
    bqt_d = nc.dram_tensor("bqt", [E, H], F32, kind="ExternalInput").ap()
    out_d = nc.dram_tensor("out", [CHUNK, D], F32, kind="ExternalOutput").ap()

    with tile.TileContext(nc) as tc, ExitStack() as ctx:
        const_p = ctx.enter_context(tc.tile_pool(name="const", bufs=1))
        big_s = ctx.enter_context(tc.tile_pool(name="bigs", bufs=1))
        wring = ctx.enter_context(tc.tile_pool(name="wring", bufs=4))
        sm_p = ctx.enter_context(tc.tile_pool(name="sm", bufs=3))
        smv_p = ctx.enter_context(tc.tile_pool(name="smv", bufs=4))
        ob_p = ctx.enter_context(tc.tile_pool(name="ob", bufs=2))
        # PSUM: 8 banks exactly: big(3) + lg(3) + st(1) + at(1)
        ps_big = ctx.enter_context(tc.tile_pool(name="ps_big", bufs=3, space="PSUM"))
        ps_lg = ctx.enter_context(tc.tile_pool(name="ps_lg", bufs=3, space="PSUM"))
        ps_st = ctx.enter_context(tc.tile_pool(name="ps_st", bufs=1, space="PSUM"))
        ps_at = ctx.enter_context(tc.tile_pool(name="ps_at", bufs=1, space="PSUM"))

        # ---- constants (Pool-generated; no DMA dependency) ----
        warmb = const_p.tile([128, 256], BF16)
        nc.gpsimd.memset(warmb, 0.0)
        identb = const_p.tile([128, 128], BF16)
        make_identity(nc, identb)

        for _rep in range(reps):
            _emit(nc, tc, mybir, F32, BF16, FP16, FR, AF,
                  xt_d, wq_d, wk_d, wvrb_d, wosb_d, mkb_d, bqt_d, out_d,
                  const_p, big_s, wring, sm_p, smv_p, ob_p,
                  ps_big, ps_lg, ps_st, ps_at, identb, warmb)

    nc.compile()
    return nc


def _emit(nc, tc, mybir, F32, BF16, FP16, FR, AF,
          xt_d, wq_d, wk_d, wvrb_d, wosb_d, mkb_d, bqt_d, out_d,
          const_p, big_s, wring, sm_p, smv_p, ob_p,
          ps_big, ps_lg, ps_st, ps_at, identb, warmb):
    # ---- resident loads, ordered by first PE use; head-0 weights and xt are
    # chunk-interleaved so the first q-projection matmul can start ~2.5us in ----
    wq0 = wring.tile([128, DC, E], FP16, tag="wq", name="wq0")
    wk0 = wring.tile([128, DC, E], FP16, tag="wk", name="wk0")
    xt_sb = big_s.tile([128, DC, SPAN], FP16, tag="xt")
    wqr = wq_d[0].rearrange("p (c e) -> p c e", c=DC)
    nc.sync.dma_start(wq0[:, 0, :], wqr[:, 0, :])
    nc.sync.dma_start(xt_sb[:, 0, :], xt_d[0:128, :])
    nc.sync.dma_start(wq0[:, 1:DC, :], wqr[:, 1:DC, :])
    for c in range(1, 4):
        nc.sync.dma_start(xt_sb[:, c, :], xt_d[128 * c:128 * (c + 1), :])
    nc.sync.dma_start(wk0, wk_d[0].rearrange("p (c e) -> p c e", c=DC))
    for c in range(4, DC):
        nc.sync.dma_start(xt_sb[:, c, :], xt_d[128 * c:128 * (c + 1), :])
    bqt_sb = big_s.tile([128, H], F32, tag="bqt")
    nc.sync.dma_start(bqt_sb, bqt_d)

    wvrb_sb = big_s.tile([128, DC, H * E], BF16, tag="wvrb")

    w_ring = {0: (wq0, wk0)}
    def _load_head(h):
        wqh = wring.tile([128, DC, E], FP16, tag="wq", name=f"wq{h}")
        nc.sync.dma_start(wqh, wq_d[h].rearrange("p (c e) -> p c e", c=DC))
        wkh = wring.tile([128, DC, E], FP16, tag="wk", name=f"wk{h}")
        nc.sync.dma_start(wkh, wk_d[h].rearrange("p (c e) -> p c e", c=DC))
        w_ring[h] = (wqh, wkh)

    _load_head(1)
    nc.sync.dma_start(wvrb_sb[:, :, 0:512],
                      wvrb_d[:, 0:512].rearrange("(c p) n -> p c n", p=128))
    nc.sync.dma_start(wvrb_sb[:, :, 512:1024],
                      wvrb_d[:, 512:1024].rearrange("(c p) n -> p c n", p=128))
    _load_head(2)
    _load_head(3)
    mkb_sb = big_s.tile([128, H, 256], BF16, tag="mkb")
    nc.sync.dma_start(mkb_sb, mkb_d.rearrange("h p t -> p h t"))
    wosb_sb = big_s.tile([128, H, D], BF16, tag="wosb")
    nc.sync.dma_start(wosb_sb, wosb_d.rearrange("h e d -> e h d"))
    for h in range(4, H):
        _load_head(h)

    # ---- persistent projection outputs ----
    qT_sb = big_s.tile([128, H, CHUNK], FP16, tag="qT")  # [e, h, s]
    kT_sb = big_s.tile([128, H, SPAN], FP16, tag="kT")   # [e, h, s]
    xtb_sb = big_s.tile([128, DC, SPAN], BF16, tag="xtb")
    vb_sb = big_s.tile([128, NST, H * E], BF16, tag="vb")  # [s, tile, h*E+e]

    # ---- PE warm-up: ramp p-state while DMAs stream (no data deps) ----
    warm_n = [0]
    def _warm(k):
        for _ in range(k):
            wp = ps_lg.tile([128, 256], F32, tag="lg", name=f"warm{warm_n[0]}")
            warm_n[0] += 1
            nc.tensor.matmul(wp, warmb[:, 0:128], warmb[:, 0:256],
                             start=True, stop=True)

    _warm(N_WARM)

    # ---- Pool setup: zero kT (edges beyond K_SPANS must be finite), cast xt
    # to bf16 for the v projection ----
    nc.gpsimd.memset(kT_sb, 0.0)
    for c in range(DC):
        nc.gpsimd.tensor_copy(xtb_sb[:, c, :], xt_sb[:, c, :])

    # ---- phase 1: projections ----
    def _qk(h, fill=0):
        wqh, wkh = w_ring[h]
        qp = ps_big.tile([128, 512], F32, tag="big", name=f"qp{h}")
        for c in range(DC):
            nc.tensor.matmul(qp, wqh[:, c, :], xt_sb[:, c, HALO:HALO + CHUNK],
                             start=(c == 0), stop=(c == DC - 1))
            if c < DC - 1:
                _warm(fill)  # cover DMA-paced gaps while xt chunks stream in
        nc.scalar.activation(qT_sb[:, h, :], qp, mybir.ActivationFunctionType.Identity,
                             bias=bqt_sb[:, h:h + 1], scale=1.0)
        # k projected only over the span this head's dilated windows can touch;
        # the rest of kT stays at the one-time memset zeros (masked out anyway)
        s0, s1 = K_SPANS[h]
        w1 = (s1 - s0) // 2
        for sl in (slice(s0, s0 + w1), slice(s0 + w1, s1)):
            kp = ps_big.tile([128, 512], F32, tag="big", name=f"kp{h}_{sl.start}")
            w = sl.stop - sl.start
            for c in range(DC):
                nc.tensor.matmul(kp[:, 0:w], wkh[:, c, :], xt_sb[:, c, sl],
                                 start=(c == 0), stop=(c == DC - 1))
            nc.scalar.copy(kT_sb[:, h, sl], kp[:, 0:w])

    def _vhalf(half):
        nsl = slice(512 * half, 512 * (half + 1))
        for j in range(NST):
            vp = ps_big.tile([128, 512], F32, tag="big", name=f"vp{half}_{j}")
            for c in range(DC):
                nc.tensor.matmul(vp, xtb_sb[:, c, 128 * j:128 * (j + 1)],
                                 wvrb_sb[:, c, nsl], start=(c == 0), stop=(c == DC - 1))
            nc.vector.tensor_copy(vb_sb[:, j, nsl], vp)

    # ---- phase 2 closures: attention, software pipelined ----
    lg_t, ex_t, se_t, rc_t, sc_t, st_t, sct_t, at_t, ats_t = ({} for _ in range(9))
    ou_t = {}

    def e_lg(t):
        i, h = divmod(t, 8)
        lg = ps_lg.tile([128, 256], F32, tag="lg", name=f"lg{t}")
        lg_t[t] = lg
        nc.tensor.matmul(lg, identb, mkb_sb[:, h, :], start=True, stop=False)
        nc.tensor.matmul(lg, qT_sb[:, h, 128 * i:128 * (i + 1)],
                         kT_sb[:, h, 128 * i:128 * i + 256],
                         start=False, stop=True)

    def e_exp(t):
        ex = sm_p.tile([128, 256], BF16, tag="ex", name=f"ex{t}")
        se = smv_p.tile([128, 1], F32, tag="se", name=f"se{t}")
        nc.scalar.activation(ex, lg_t.pop(t), AF.Exp, bias=0.0, scale=1.0,
                             accum_out=se)
        ex_t[t], se_t[t] = ex, se

    def e_recip(t):
        rc = smv_p.tile([128, 1], F32, tag="rc", name=f"rc{t}")
        nc.vector.reciprocal(rc, se_t.pop(t))
        rc_t[t] = rc

    def e_mul(t):
        sc = sm_p.tile([128, 256], BF16, tag="sc", name=f"sc{t}")
        nc.gpsimd.tensor_scalar_mul(sc, ex_t.pop(t), rc_t.pop(t))
        sc_t[t] = sc

    def e_tr(t):
        st = ps_st.tile([128, 256], BF16, tag="st", name=f"st{t}")
        sc = sc_t.pop(t)
        nc.tensor.transpose(st[:, 0:128], sc[:, 0:128], identb)
        nc.tensor.transpose(st[:, 128:256], sc[:, 128:256], identb)
        st_t[t] = st

    def e_sct(t):
        sct = sm_p.tile([128, 256], BF16, tag="sct", name=f"sct{t}")
        nc.vector.tensor_copy(sct, st_t.pop(t))
        sct_t[t] = sct

    def e_at(t):
        i, h = divmod(t, 8)
        at = ps_at.tile([128, 128], F32, tag="at", name=f"at{t}")
        sct = sct_t.pop(t)
        nc.tensor.matmul(at, vb_sb[:, i, E * h:E * (h + 1)], sct[:, 0:128],
                         start=True, stop=False)
        nc.tensor.matmul(at, vb_sb[:, i + 1, E * h:E * (h + 1)], sct[:, 128:256],
                         start=False, stop=True)
        at_t[t] = at

    def e_ats(t):
        ats = sm_p.tile([128, 128], BF16, tag="ats", name=f"ats{t}")
        nc.vector.tensor_copy(ats, at_t.pop(t))
        ats_t[t] = ats

    def e_op(t):
        i, h = divmod(t, 8)
        if h == 0:
            ou0 = ps_big.tile([128, 512], F32, tag="big", name=f"ou0_{i}")
            ou1 = ps_big.tile([128, 512], F32, tag="big", name=f"ou1_{i}")
            ou_t[i] = (ou0, ou1)
        ou0, ou1 = ou_t[i]
        ats = ats_t.pop(t)
        nc.tensor.matmul(ou0, ats, wosb_sb[:, h, 0:512],
                         start=(h == 0), stop=(h == 7))
        nc.tensor.matmul(ou1, ats, wosb_sb[:, h, 512:1024],
                         start=(h == 0), stop=(h == 7))

    def e_ob(i):
        ou0, ou1 = ou_t.pop(i)
        ob = ob_p.tile([128, D], F32, tag="ob", name=f"ob{i}")
        nc.scalar.copy(ob[:, 0:512], ou0)
        nc.sync.dma_start(out_d[128 * i:128 * (i + 1), 0:512], ob[:, 0:512])
        if i >= 2:
            # late tiles: second half on DVE so ACT stays free for drain exps
            nc.vector.tensor_copy(ob[:, 512:1024], ou1)
        else:
            nc.scalar.copy(ob[:, 512:1024], ou1)
        nc.sync.dma_start(out_d[128 * i:128 * (i + 1), 512:1024], ob[:, 512:1024])

    # ---- emission: projections with the phase-2 prologue overlapped into the
    # tail of phase 1 (softmax chain of tiles 0-1 runs while head 7 projects) ----
    _qk(0, fill=QK0_FILL)
    _qk(1)
    _vhalf(0)
    _qk(2)
    _vhalf(1)
    _qk(3)
    _qk(4)
    _qk(5)
    _qk(6)
    e_lg(0)
    e_exp(0)
    e_recip(0)
    e_mul(0)
    e_lg(1)
    _qk(7)
    e_exp(1)
    e_recip(1)
    e_mul(1)
    e_lg(2)
    e_exp(2)
    e_recip(2)
    e_mul(2)
    e_tr(0)
    e_sct(0)
    PRE_CHAIN, PRE_TR = 3, 1

    # pipeline, slot u: PE [tr(u-2), at(u-3), op(u-4), lg(u+2)],
    # ACT [exp(u), ob], DVE [sct(u-2), ats(u-3), recip(u)], Pool [mul(u)].
    # Chain lg(t)->exp(t)->recip(t)->mul(t) finishes mid slot t+1; tr(t) runs
    # slot t+2, so PE never waits on the softmax chain in steady state.
    for u in range(NT + 6):
        if PRE_TR <= u - 2 < NT:
            e_tr(u - 2)
        if 0 <= u - 3 < NT:
            e_at(u - 3)
        if 0 <= u - 4 < NT:
            e_op(u - 4)
        if PRE_CHAIN <= u + 2 < NT:
            e_lg(u + 2)
        if PRE_CHAIN <= u < NT:
            e_exp(u)
        if u >= 11 and (u - 11) % 8 == 0 and (u - 11) // 8 < NQT:
            e_ob((u - 11) // 8)
        if PRE_TR <= u - 2 < NT:
            e_sct(u - 2)
        if 0 <= u - 3 < NT:
            e_ats(u - 3)
        if PRE_CHAIN <= u < NT:
            e_recip(u)
            e_mul(u)


def _host_prep(x, Wq, bq, Wk, bk, Wv, bv, Wo, bo, dilations):
    import ml_dtypes
    f = np.float32
    bf = ml_dtypes.bfloat16
    x = np.asarray(x, f)
    x_pad = np.zeros((B, S + 2 * HALO, D), f)
    x_pad[:, HALO:HALO + S] = x

    Wo_s = np.asarray(Wo, f) * np.float32(E) ** f(-0.5)
    wvrb = np.ascontiguousarray(
        np.asarray(Wv, f).transpose(1, 0, 2).reshape(D, H * E)).astype(bf)
    wosb = np.ascontiguousarray(Wo_s).astype(bf)
    bqt = np.ascontiguousarray(np.asarray(bq, f).T)      # [E, H]

    # host-folded constant: sum_h (bv_h/sqrt(E)) @ Wo_h + bo  (sum of scores = 1)
    hostc = np.einsum('he,hed->d', np.asarray(bv, f) * np.float32(E) ** f(-0.5),
                      np.asarray(Wo, f)) + np.asarray(bo, f)

    dil = np.asarray(dilations).astype(np.int64)
    masks = np.full((H, 128, 256), MASKVAL, f)
    s_i = np.arange(128)[:, None]
    t_i = np.arange(256)[None, :]
    for h in range(H):
        d = int(dil[h])
        off = (d * (KW - 1)) // 2
        delta = t_i - s_i - HALO + off
        win = (delta >= 0) & (delta <= (KW - 1) * d) & (delta % d == 0)
        masks[h][win] = SHIFT
    mkb = masks.astype(bf)

    # q/k path in fp16, weights pre-rearranged to [H, 128, DC*E] (contiguous
    # >=512B DMA rows: [p, c, e] layout per head)
    wq16 = np.ascontiguousarray(
        np.asarray(Wq, f).reshape(H, DC, 128, E).transpose(0, 2, 1, 3)
        .reshape(H, 128, DC * E)).astype(np.float16)
    wk16 = np.ascontiguousarray(
        np.asarray(Wk, f).reshape(H, DC, 128, E).transpose(0, 2, 1, 3)
        .reshape(H, 128, DC * E)).astype(np.float16)

    shared = {
        "wq": wq16, "wk": wk16,
        "wvrb": wvrb, "wosb": wosb, "mkb": mkb, "bqt": bqt,
    }
    in_maps = []
    for c in range(NC_):
        b, idx = divmod(c, 4)
        xt = np.ascontiguousarray(
            x_pad[b, idx * CHUNK: idx * CHUNK + SPAN].T).astype(np.float16)
        in_maps.append({"xt": xt, **shared})
    return in_maps, hostc


def kernel(x, Wq, bq, Wk, bk, Wv, bv, Wo, bo, dilations):
    from concourse.bass_utils import run_bass_kernel_spmd

    if "nc" not in _CACHE:
        _CACHE["nc"] = _build_nc()
    nc = _CACHE["nc"]

    in_maps, hostc = _host_prep(x, Wq, bq, Wk, bk, Wv, bv, Wo, bo, dilations)
    res = run_bass_kernel_spmd(nc, in_maps, core_ids=list(range(NC_)))

    out = np.empty((B, S, D), np.float32)
    for c in range(NC_):
        b, idx = divmod(c, 4)
        out[b, idx * CHUNK:(idx + 1) * CHUNK] = res.results[c]["out"]
    out += hostc[None, None, :]
    return out


# revision 42
# speedup vs baseline: 1.5847x; 1.0063x over previous
"""LCSA (local convolutional sparse attention) Trainium2 Bass kernel.

Problem: B=2, S=2048, D=1024, H=8 heads, E=128 head width, KW=16 kernel width,
per-head dilations [1,1,2,2,4,4,8,8].

Sharding: data-parallel over (batch, sequence): core c handles batch c//4,
sequence chunk (c%4)*512..+512, with a 64-token zero-padded halo per side.

Device algorithm per core (fp32 q/k path, bf16 value path):
  - qT[h] = Wq[h].T @ xT [E,512]; kT[h] [E,640] (fp32r, PE).  k-bias dropped
    (uniform per-query logit shift -> softmax invariant); q-bias via ACT copy.
  - v = xTb.T @ Wv_allheads [640, H*E] in bf16 (xt cast to bf16 on Pool).
    v-bias and out-bias folded into a host-side constant (sum of scores = 1).
  - Per (query tile i, head h): PSUM logits = mask (PE identity-matmul
    preload, in-window value -40 to bound exp) + qT_i.T @ kT window [128,256];
    exp+rowsum on ACT (no max subtraction; |logit|<=81 so exp(l-40) is safe);
    reciprocal on DVE; normalize on Pool (bf16); transpose via PE (bf16);
    attnT = v.T @ scoreT (bf16); out_i = sum_h attnT.T @ Wo[h] (bf16 moving).
  - Software-pipelined emission keeps all five engines concurrently busy;
    warm-up matmuls ramp the PE p-state before real data lands.
"""

import numpy as np

B, S, D, H, E, KW = 2, 2048, 1024, 8, 128, 16
HALO = 64          # covers max offset d*(KW-1)//2 = 60 for d=8
CHUNK = 512        # query tokens per core
SPAN = CHUNK + 2 * HALO   # 640 kv tokens per core
NST = SPAN // 128  # 5 sequence tiles
NQT = CHUNK // 128 # 4 query tiles
NC_ = 8            # cores
DC = D // 128      # 8 contraction chunks
NT = NQT * H       # 32 attention tiles per core
MASKVAL = -30000.0
SHIFT = -40.0      # in-window logit shift; bounds exp while leaving softmax exact

_CACHE: dict = {}
N_WARM = 8         # PE warm-up matmuls (p-state ramp + DMA-latency cover)
SEAM_FILL = 4      # fillers at phase-1 seams to bridge DMA waits (p-state)
QK0_FILL = 1        # filler matmuls between head-0 q chunks (xt DMA pacing)
DILATIONS = (1, 1, 2, 2, 4, 4, 8, 8)
# per-head kv span (in 640-wide span coords) actually reachable by the windows
K_SPANS = tuple((HALO - (15 * d) // 2, HALO + CHUNK + 15 * d - (15 * d) // 2)
                for d in DILATIONS)
# per-head logits window width from 128*i (span coords), multiple of 8, <=256
W_H = tuple(min(256, (HALO + 128 + 15 * d - (15 * d) // 2 + 7) // 8 * 8)
            for d in DILATIONS)


def _build_nc(reps=1, f32r=True):
    from contextlib import ExitStack

    import concourse.bacc as bacc
    import concourse.tile as tile
    from concourse import mybir
    from concourse.masks import make_identity

    F32 = mybir.dt.float32
    BF16 = mybir.dt.bfloat16
    FP16 = mybir.dt.float16
    FR = mybir.dt.float32r if f32r else F32
    AF = mybir.ActivationFunctionType

    nc = bacc.Bacc("TRN2", target_bir_lowering=False, debug=False, num_devices=1)

    # q/k path in fp16 (4x finer mantissa than bf16; halves the weight DMA).
    # wq/wk pre-rearranged on host to [H, 128, DC*E] so DMA rows stay >=512B.
    xt_d = nc.dram_tensor("xt", [D, SPAN], FP16, kind="ExternalInput").ap()
    wq_d = nc.dram_tensor("wq", [H, 128, DC * E], FP16, kind="ExternalInput").ap()
    wk_d = nc.dram_tensor("wk", [H, 128, DC * E], FP16, kind="ExternalInput").ap()
    wvrb_d = nc.dram_tensor("wvrb", [D, H * E], BF16, kind="ExternalInput").ap()
    wosb_d = nc.dram_tensor("wosb", [H, E, D], BF16, kind="ExternalInput").ap()
    mkb_d = nc.dram_tensor("mkb", [H, 128, 256], BF16, kind="ExternalInput").ap()
    bqt_d = nc.dram_tensor("bqt", [E, H], F32, kind="ExternalInput").ap()
    out_d = nc.dram_tensor("out", [CHUNK, D], F32, kind="ExternalOutput").ap()

    with tile.TileContext(nc) as tc, ExitStack() as ctx:
        const_p = ctx.enter_context(tc.tile_pool(name="const", bufs=1))
        big_s = ctx.enter_context(tc.tile_pool(name="bigs", bufs=1))
        wring = ctx.enter_context(tc.tile_pool(name="wring", bufs=4))
        sm_p = ctx.enter_context(tc.tile_pool(name="sm", bufs=3))
        smv_p = ctx.enter_context(tc.tile_pool(name="smv", bufs=4))
        ob_p = ctx.enter_context(tc.tile_pool(name="ob", bufs=2))
        # PSUM: 8 banks exactly: big(3) + lg(3) + st(1) + at(1)
        ps_big = ctx.enter_context(tc.tile_pool(name="ps_big", bufs=3, space="PSUM"))
        ps_lg = ctx.enter_context(tc.tile_pool(name="ps_lg", bufs=3, space="PSUM"))
        ps_st = ctx.enter_context(tc.tile_pool(name="ps_st", bufs=1, space="PSUM"))
        ps_at = ctx.enter_context(tc.tile_pool(name="ps_at", bufs=1, space="PSUM"))

        # ---- constants (Pool-generated; no DMA dependency) ----
        warmb = const_p.tile([128, 256], BF16)
        nc.gpsimd.memset(warmb, 0.0)
        identb = const_p.tile([128, 128], BF16)
        make_identity(nc, identb)

        for _rep in range(reps):
            _emit(nc, tc, mybir, F32, BF16, FP16, FR, AF,
                  xt_d, wq_d, wk_d, wvrb_d, wosb_d, mkb_d, bqt_d, out_d,
                  const_p, big_s, wring, sm_p, smv_p, ob_p,
                  ps_big, ps_lg, ps_st, ps_at, identb, warmb)

    nc.compile()
    return nc


def _emit(nc, tc, mybir, F32, BF16, FP16, FR, AF,
          xt_d, wq_d, wk_d, wvrb_d, wosb_d, mkb_d, bqt_d, out_d,
          const_p, big_s, wring, sm_p, smv_p, ob_p,
          ps_big, ps_lg, ps_st, ps_at, identb, warmb):
    # ---- resident loads, ordered by first PE use; head-0 weights and xt are
    # chunk-interleaved so the first q-projection matmul can start ~2.5us in ----
    wq0 = wring.tile([128, DC, E], FP16, tag="wq", name="wq0")
    wk0 = wring.tile([128, DC, E], FP16, tag="wk", name="wk0")
    xt_sb = big_s.tile([128, DC, SPAN], FP16, tag="xt")
    wqr = wq_d[0].rearrange("p (c e) -> p c e", c=DC)
    nc.sync.dma_start(wq0[:, 0, :], wqr[:, 0, :])
    nc.sync.dma_start(xt_sb[:, 0, :], xt_d[0:128, :])
    nc.sync.dma_start(wq0[:, 1:DC, :], wqr[:, 1:DC, :])
    for c in range(1, 4):
        nc.sync.dma_start(xt_sb[:, c, :], xt_d[128 * c:128 * (c + 1), :])
    nc.sync.dma_start(wk0, wk_d[0].rearrange("p (c e) -> p c e", c=DC))
    for c in range(4, DC):
        nc.sync.dma_start(xt_sb[:, c, :], xt_d[128 * c:128 * (c + 1), :])
    bqt_sb = big_s.tile([128, H], F32, tag="bqt")
    nc.sync.dma_start(bqt_sb, bqt_d)

    wvrb_sb = big_s.tile([128, DC, H * E], BF16, tag="wvrb")

    w_ring = {0: (wq0, wk0)}
    def _load_head(h):
        wqh = wring.tile([128, DC, E], FP16, tag="wq", name=f"wq{h}")
        nc.sync.dma_start(wqh, wq_d[h].rearrange("p (c e) -> p c e", c=DC))
        wkh = wring.tile([128, DC, E], FP16, tag="wk", name=f"wk{h}")
        nc.sync.dma_start(wkh, wk_d[h].rearrange("p (c e) -> p c e", c=DC))
        w_ring[h] = (wqh, wkh)

    _load_head(1)
    nc.sync.dma_start(wvrb_sb[:, :, 0:512],
                      wvrb_d[:, 0:512].rearrange("(c p) n -> p c n", p=128))
    nc.sync.dma_start(wvrb_sb[:, :, 512:1024],
                      wvrb_d[:, 512:1024].rearrange("(c p) n -> p c n", p=128))
    _load_head(2)
    _load_head(3)
    mkb_sb = big_s.tile([128, H, 256], BF16, tag="mkb")
    nc.sync.dma_start(mkb_sb, mkb_d.rearrange("h p t -> p h t"))
    wosb_sb = big_s.tile([128, H, D], BF16, tag="wosb")
    nc.sync.dma_start(wosb_sb, wosb_d.rearrange("h e d -> e h d"))
    for h in range(4, H):
        _load_head(h)

    # ---- persistent projection outputs ----
    qT_sb = big_s.tile([128, H, CHUNK], FP16, tag="qT")  # [e, h, s]
    kT_sb = big_s.tile([128, H, SPAN], FP16, tag="kT")   # [e, h, s]
    xtb_sb = big_s.tile([128, DC, SPAN], BF16, tag="xtb")
    vb_sb = big_s.tile([128, NST, H * E], BF16, tag="vb")  # [s, tile, h*E+e]

    # ---- PE warm-up: ramp p-state while DMAs stream (no data deps) ----
    warm_n = [0]
    def _warm(k):
        for _ in range(k):
            wp = ps_lg.tile([128, 256], F32, tag="lg", name=f"warm{warm_n[0]}")
            warm_n[0] += 1
            nc.tensor.matmul(wp, warmb[:, 0:128], warmb[:, 0:256],
                             start=True, stop=True)

    _warm(N_WARM)

    # ---- Pool setup: zero kT (edges beyond K_SPANS must be finite), cast xt
    # to bf16 for the v projection ----
    nc.gpsimd.memset(kT_sb, 0.0)
    for c in range(DC):
        nc.gpsimd.tensor_copy(xtb_sb[:, c, :], xt_sb[:, c, :])

    # ---- phase 1: projections ----
    def _qk(h, fill=0):
        wqh, wkh = w_ring[h]
        qp = ps_big.tile([128, 512], F32, tag="big", name=f"qp{h}")
        for c in range(DC):
            nc.tensor.matmul(qp, wqh[:, c, :], xt_sb[:, c, HALO:HALO + CHUNK],
                             start=(c == 0), stop=(c == DC - 1))
            if c < DC - 1:
                _warm(fill)  # cover DMA-paced gaps while xt chunks stream in
        nc.scalar.activation(qT_sb[:, h, :], qp, mybir.ActivationFunctionType.Identity,
                             bias=bqt_sb[:, h:h + 1], scale=1.0)
        # k projected only over the span this head's dilated windows can touch;
        # the rest of kT stays at the one-time memset zeros (masked out anyway)
        s0, s1 = K_SPANS[h]
        w1 = (s1 - s0) // 2
        for sl in (slice(s0, s0 + w1), slice(s0 + w1, s1)):
            kp = ps_big.tile([128, 512], F32, tag="big", name=f"kp{h}_{sl.start}")
            w = sl.stop - sl.start
            for c in range(DC):
                nc.tensor.matmul(kp[:, 0:w], wkh[:, c, :], xt_sb[:, c, sl],
                                 start=(c == 0), stop=(c == DC - 1))
            nc.scalar.copy(kT_sb[:, h, sl], kp[:, 0:w])

    def _vhalf(half):
        nsl = slice(512 * half, 512 * (half + 1))
        for j in range(NST):
            vp = ps_big.tile([128, 512], F32, tag="big", name=f"vp{half}_{j}")
            for c in range(DC):
                nc.tensor.matmul(vp, xtb_sb[:, c, 128 * j:128 * (j + 1)],
                                 wvrb_sb[:, c, nsl], start=(c == 0), stop=(c == DC - 1))
            nc.vector.tensor_copy(vb_sb[:, j, nsl], vp)

    # ---- phase 2 closures: attention, software pipelined ----
    lg_t, ex_t, se_t, rc_t, sc_t, st_t, sct_t, at_t, ats_t = ({} for _ in range(9))
    ou_t = {}

    def e_lg(t):
        i, h = divmod(t, 8)
        lg = ps_lg.tile([128, 256], F32, tag="lg", name=f"lg{t}")
        lg_t[t] = lg
        nc.tensor.matmul(lg, identb, mkb_sb[:, h, :], start=True, stop=False)
        nc.tensor.matmul(lg, qT_sb[:, h, 128 * i:128 * (i + 1)],
                         kT_sb[:, h, 128 * i:128 * i + 256],
                         start=False, stop=True)

    def e_exp(t):
        ex = sm_p.tile([128, 256], BF16, tag="ex", name=f"ex{t}")
        se = smv_p.tile([128, 1], F32, tag="se", name=f"se{t}")
        nc.scalar.activation(ex, lg_t.pop(t), AF.Exp, bias=0.0, scale=1.0,
                             accum_out=se)
        ex_t[t], se_t[t] = ex, se

    def e_recip(t):
        rc = smv_p.tile([128, 1], F32, tag="rc", name=f"rc{t}")
        nc.vector.reciprocal(rc, se_t.pop(t))
        rc_t[t] = rc

    def e_mul(t):
        sc = sm_p.tile([128, 256], BF16, tag="sc", name=f"sc{t}")
        nc.gpsimd.tensor_scalar_mul(sc, ex_t.pop(t), rc_t.pop(t))
        sc_t[t] = sc

    def e_tr(t):
        st = ps_st.tile([128, 256], BF16, tag="st", name=f"st{t}")
        sc = sc_t.pop(t)
        nc.tensor.transpose(st[:, 0:128], sc[:, 0:128], identb)
        nc.tensor.transpose(st[:, 128:256], sc[:, 128:256], identb)
        st_t[t] = st

    def e_sct(t):
        sct = sm_p.tile([128, 256], BF16, tag="sct", name=f"sct{t}")
        nc.vector.tensor_copy(sct, st_t.pop(t))
        sct_t[t] = sct

    def e_at(t):
        i, h = divmod(t, 8)
        at = ps_at.tile([128, 128], F32, tag="at", name=f"at{t}")
        sct = sct_t.pop(t)
        nc.tensor.matmul(at, vb_sb[:, i, E * h:E * (h + 1)], sct[:, 0:128],
                         start=True, stop=False)
        nc.tensor.matmul(at, vb_sb[:, i + 1, E * h:E * (h + 1)], sct[:, 128:256],
                         start=False, stop=True)
        at_t[t] = at

    def e_ats(t):
        ats = sm_p.tile([128, 128], BF16, tag="ats", name=f"ats{t}")
        nc.vector.tensor_copy(ats, at_t.pop(t))
        ats_t[t] = ats

    def e_op(t):
        i, h = divmod(t, 8)
        if h == 0:
            ou0 = ps_big.tile([128, 512], F32, tag="big", name=f"ou0_{i}")
            ou1 = ps_big.tile([128, 512], F32, tag="big", name=f"ou1_{i}")
            ou_t[i] = (ou0, ou1)
        ou0, ou1 = ou_t[i]
        ats = ats_t.pop(t)
        nc.tensor.matmul(ou0, ats, wosb_sb[:, h, 0:512],
                         start=(h == 0), stop=(h == 7))
        nc.tensor.matmul(ou1, ats, wosb_sb[:, h, 512:1024],
                         start=(h == 0), stop=(h == 7))

    def e_ob(i):
        # first half on DVE (emitted at slot start so the ou bank frees fast,
        # unblocking tile i+1's first out-proj matmul), second half on ACT
        ou0, ou1 = ou_t.pop(i)
        ob = ob_p.tile([128, D], F32, tag="ob", name=f"ob{i}")
        nc.vector.tensor_copy(ob[:, 0:512], ou0)
        nc.sync.dma_start(out_d[128 * i:128 * (i + 1), 0:512], ob[:, 0:512])
        nc.scalar.copy(ob[:, 512:1024], ou1)
        nc.sync.dma_start(out_d[128 * i:128 * (i + 1), 512:1024], ob[:, 512:1024])

    # ---- emission: projections with the phase-2 prologue overlapped into the
    # tail of phase 1 (softmax chain of tiles 0-1 runs while head 7 projects) ----
    _qk(0, fill=QK0_FILL)
    _warm(SEAM_FILL)
    _qk(1)
    _warm(SEAM_FILL)
    _vhalf(0)
    _qk(2)
    _vhalf(1)
    _qk(3)
    _qk(4)
    _qk(5)
    _qk(6)
    e_lg(0)
    e_exp(0)
    e_recip(0)
    e_mul(0)
    e_lg(1)
    _qk(7)
    e_exp(1)
    e_recip(1)
    e_mul(1)
    e_lg(2)
    e_exp(2)
    e_recip(2)
    e_mul(2)
    e_tr(0)
    e_sct(0)
    e_tr(1)
    e_at(0)
    e_sct(1)
    e_ats(0)
    PRE_CHAIN, PRE_TR, PRE_AT = 3, 2, 1

    # pipeline, slot u: PE [tr(u-2), at(u-3), op(u-4), lg(u+2)],
    # ACT [exp(u), ob], DVE [sct(u-2), ats(u-3), recip(u)], Pool [mul(u)].
    # Chain lg(t)->exp(t)->recip(t)->mul(t) finishes mid slot t+1; tr(t) runs
    # slot t+2, so PE never waits on the softmax chain in steady state.
    for u in range(NT + 6):
        if PRE_TR <= u - 2 < NT:
            e_tr(u - 2)
        if PRE_AT <= u - 3 < NT:
            e_at(u - 3)
        if 0 <= u - 4 < NT:
            e_op(u - 4)
        if PRE_CHAIN <= u + 2 < NT:
            e_lg(u + 2)
        if u >= 12 and (u - 12) % 8 == 0 and (u - 12) // 8 < NQT:
            e_ob((u - 12) // 8)
        if PRE_CHAIN <= u < NT:
            e_exp(u)
        if PRE_TR <= u - 2 < NT:
            e_sct(u - 2)
        if PRE_AT <= u - 3 < NT:
            e_ats(u - 3)
        if PRE_CHAIN <= u < NT:
            e_recip(u)
            e_mul(u)


def _host_prep(x, Wq, bq, Wk, bk, Wv, bv, Wo, bo, dilations):
    import ml_dtypes
    f = np.float32
    bf = ml_dtypes.bfloat16
    x = np.asarray(x, f)
    x_pad = np.zeros((B, S + 2 * HALO, D), f)
    x_pad[:, HALO:HALO + S] = x

    Wo_s = np.asarray(Wo, f) * np.float32(E) ** f(-0.5)
    wvrb = np.ascontiguousarray(
        np.asarray(Wv, f).transpose(1, 0, 2).reshape(D, H * E)).astype(bf)
    wosb = np.ascontiguousarray(Wo_s).astype(bf)
    bqt = np.ascontiguousarray(np.asarray(bq, f).T)      # [E, H]

    # host-folded constant: sum_h (bv_h/sqrt(E)) @ Wo_h + bo  (sum of scores = 1)
    hostc = np.einsum('he,hed->d', np.asarray(bv, f) * np.float32(E) ** f(-0.5),
                      np.asarray(Wo, f)) + np.asarray(bo, f)

    dil = np.asarray(dilations).astype(np.int64)
    masks = np.full((H, 128, 256), MASKVAL, f)
    s_i = np.arange(128)[:, None]
    t_i = np.arange(256)[None, :]
    for h in range(H):
        d = int(dil[h])
        off = (d * (KW - 1)) // 2
        delta = t_i - s_i - HALO + off
        win = (delta >= 0) & (delta <= (KW - 1) * d) & (delta % d == 0)
        masks[h][win] = SHIFT
    mkb = masks.astype(bf)

    # q/k path in fp16, weights pre-rearranged to [H, 128, DC*E] (contiguous
    # >=512B DMA rows: [p, c, e] layout per head)
    wq16 = np.ascontiguousarray(
        np.asarray(Wq, f).reshape(H, DC, 128, E).transpose(0, 2, 1, 3)
        .reshape(H, 128, DC * E)).astype(np.float16)
    wk16 = np.ascontiguousarray(
        np.asarray(Wk, f).reshape(H, DC, 128, E).transpose(0, 2, 1, 3)
        .reshape(H, 128, DC * E)).astype(np.float16)

    shared = {
        "wq": wq16, "wk": wk16,
        "wvrb": wvrb, "wosb": wosb, "mkb": mkb, "bqt": bqt,
    }
    in_maps = []
    for c in range(NC_):
        b, idx = divmod(c, 4)
        xt = np.ascontiguousarray(
            x_pad[b, idx * CHUNK: idx * CHUNK + SPAN].T).astype(np.float16)
        in_maps.append({"xt": xt, **shared})
    return in_maps, hostc


def kernel(x, Wq, bq, Wk, bk, Wv, bv, Wo, bo, dilations):
    from concourse.bass_utils import run_bass_kernel_spmd

    if "nc" not in _CACHE:
        _CACHE["nc"] = _build_nc()
    nc = _CACHE["nc"]

    in_maps, hostc = _host_prep(x, Wq, bq, Wk, bk, Wv, bv, Wo, bo, dilations)
    res = run_bass_kernel_spmd(nc, in_maps, core_ids=list(range(NC_)))

    out = np.empty((B, S, D), np.float32)
    for c in range(NC_):
        b, idx = divmod(c, 4)
        out[b, idx * CHUNK:(idx + 1) * CHUNK] = res.results[c]["out"]
    out += hostc[None, None, :]
    return out
